# revision 8
# baseline (speedup 1.0000x reference)
"""MaxUnpooling2D scatter-add kernel for Trainium2 (8 NeuronCores).

Reference semantics (per batch b):
    y = mask // (OW*C); x = (mask // C) % OW; f = channel index c
    out[b, y, x, c] += updates[b, h, w, c]      (duplicates sum)

Strategy (pure data-parallel over batch; 2 batches per core):
  - One-hot matmul routing per (plane c, q-group): psum[y, x] += A_q.T @ B_q
    where A_q[i, y] = onehot(Y_i) (stationary) and B_q[i, x] = onehot(X_i)*V_i
    (moving). PSUM accumulates the 32 q-groups of a plane; duplicates sum.
  - A-tiles: DVE is_equal against a materialized iota, [P, y, q] layout
    (q innermost keeps the 2x packed mode; the strided LDWEIGHTS this causes
    overlaps under the matmuls).
  - B-tiles built two ways, split across engines for balance:
      * GPSIMD local_scatter ('g' planes): scatters V directly into the
        zeroed 1024-slot chunk at idx = (q%8)*128 + X; contiguous for PE.
      * DVE ('s' planes): xeq = is_equal + mult (both 2x packed) in the
        strided [P, x, q] layout; PE pays a strided-moving penalty but DVE
        stays at 2x throughout.
  - Output: accumulate all 128 planes of a batch into a [P(y), OW*C] fp16
    SBUF buffer (ACT evacuates psum[y, x] into column slot x*C + c), then a
    single contiguous 4 MB DMA per batch ([128, 32KB] rows). This replaces
    the v1 per-32-plane strided DMA that generated 131k x 128B packets
    (61% dynamic-DMA active, SBUF port contention, power throttle).
    Output is fp16 on device; host upconverts to fp32 during unshard.
"""

import sys

sys.path.insert(0, "/opt/trn_rl_repo")

import numpy as np

import concourse.bacc as bacc
import concourse.tile as tile
from concourse import mybir, library_config
from concourse.bass_utils import run_bass_kernel_spmd

# Problem shape (hardcoded per contract)
B, H, W, C = 16, 64, 64, 128
OH, OW = 2 * H, 2 * W
N_CORES = 8
B_PER_CORE = B // N_CORES  # 2
HWF = H * W  # 4096
P = 128
Q = HWF // P  # 32 hw rows per partition
NCOL = Q * C  # 4096
NHALF = NCOL // 2  # decode in halves to save SBUF
QH = Q // 2
OWC = OW * C  # 16384

F32 = mybir.dt.float32
FP16 = mybir.dt.float16
I32 = mybir.dt.int32
I16 = mybir.dt.int16

# plane classes by c % 32, finely interleaved so no engine starves:
# 'g' GPSIMD local_scatter (contiguous B), 's' DVE 2x builds with strided B
# (slower MMs)
S_SET = frozenset({1, 4, 7, 10, 13, 16, 19, 22, 25, 28, 31})

# local_scatter q-chunking: (q0, nq, num_elems); idx = (q - q0)*128 + X.
LS_CHUNKS = ((0, 14, 1792), (14, 14, 1792), (28, 4, 512))


def build_nc(s_set=S_SET):
    nc = bacc.Bacc("TRN2", target_bir_lowering=False, debug=False)

    upd = nc.declare_dram_parameter("updates", [B_PER_CORE, HWF, C], F32, isOutput=False)
    msk = nc.declare_dram_parameter("mask", [B_PER_CORE, HWF, C], I32, isOutput=False)
    iota_in = nc.declare_dram_parameter("iota", [P, P], F32, isOutput=False)
    jq_in = nc.declare_dram_parameter("jq", [P, Q], I32, isOutput=False)
    out = nc.declare_dram_parameter("out", [B_PER_CORE, OH, OWC], FP16, isOutput=True)

    def plane_class(c):
        return "s" if (c % 32) in s_set else "g"

    with tile.TileContext(nc) as tc:
        with (
            tc.tile_pool(name="const", bufs=1) as const_pool,
            tc.tile_pool(name="dec", bufs=1) as dec_pool,
            tc.tile_pool(name="tr", bufs=2) as tr_pool,
            tc.tile_pool(name="osb", bufs=1) as out_pool,
            tc.tile_pool(name="apool", bufs=2) as a_pool,
            tc.tile_pool(name="bpool", bufs=2) as b_pool,
            tc.tile_pool(name="bpool2", bufs=2) as b_pool2,
            tc.tile_pool(name="xpool", bufs=1) as x_pool,
            tc.tile_pool(name="cpool", bufs=2) as c_pool,
            tc.tile_pool(name="psum", bufs=8, space="PSUM") as psum_pool,
        ):
            nc.gpsimd.load_library(library_config.local_scatter)

            iota_f = const_pool.tile([P, P], F32)
            nc.sync.dma_start(iota_f[:], iota_in[:])
            jq = const_pool.tile([P, Q], I32)
            nc.sync.dma_start(jq[:], jq_in[:])
            # iotaT[p, y, q] = y (fp16, innermost step 1 -> DVE 2x packed)
            iotaT = const_pool.tile([P, P, Q], FP16)
            nc.vector.tensor_copy(
                iotaT[:],
                iota_f[:].rearrange("p (y o) -> p y o", o=1).broadcast_to([P, P, Q]),
            )

            for b in range(B_PER_CORE):
                # ---- load + decode batch b (in column-halves) ----
                ytr = tr_pool.tile([P, C, Q], FP16, tag="ytr")
                vtr = tr_pool.tile([P, C, Q], FP16, tag="vtr")
                xtr = tr_pool.tile([P, C, Q], FP16, tag="xtr")
                idx16 = tr_pool.tile([P, C, Q], I16, tag="idx16")
                for h in range(2):
                    qs = slice(h * QH, (h + 1) * QH)
                    cs = slice(h * NHALF, (h + 1) * NHALF)
                    u_f = dec_pool.tile([P, NHALF], F32, tag="uf")
                    nc.sync.dma_start(
                        u_f[:], upd[b].rearrange("(p q) c -> p (q c)", p=P)[:, cs]
                    )
                    m = dec_pool.tile([P, NHALF], I32, tag="m")
                    nc.sync.dma_start(
                        m[:], msk[b].rearrange("(p q) c -> p (q c)", p=P)[:, cs]
                    )
                    nc.scalar.copy(
                        vtr[:, :, qs], u_f[:].rearrange("p (q c) -> p c q", c=C)
                    )
                    yi = dec_pool.tile([P, NHALF], I32, tag="yi")
                    nc.vector.tensor_scalar(
                        yi[:], m[:], 14, None, mybir.AluOpType.logical_shift_right
                    )
                    nc.scalar.copy(
                        ytr[:, :, qs], yi[:].rearrange("p (q c) -> p c q", c=C)
                    )
                    xi = dec_pool.tile([P, NHALF], I32, tag="xi")
                    nc.vector.tensor_scalar(
                        xi[:],
                        m[:],
                        7,
                        127,
                        mybir.AluOpType.logical_shift_right,
                        mybir.AluOpType.bitwise_and,
                    )
                    nc.scalar.copy(
                        xtr[:, :, qs], xi[:].rearrange("p (q c) -> p c q", c=C)
                    )
                    # idx16[p, c, q] = X + (q%8)*128 (int16, for local_scatter)
                    nc.vector.scalar_tensor_tensor(
                        idx16[:, :, qs],
                        xi[:].rearrange("p (q c) -> p c q", c=C),
                        0,
                        jq[:, qs]
                        .rearrange("p (o q) -> p o q", o=1)
                        .broadcast_to([P, C, QH]),
                        mybir.AluOpType.add,
                        mybir.AluOpType.add,
                    )

                # [P(y), OW*C] fp16 accumulator for the whole batch output
                out_sb = out_pool.tile([P, OWC], FP16, tag="osb")
                osb_v = out_sb[:].rearrange("p (x c) -> p x c", c=C)

                for c in range(C):
                    # stationary: a[p, y, q] = (iotaT == Y) -- DVE 2x packed
                    a_pl = a_pool.tile([P, P, Q], FP16, tag="a")
                    y_bc = (
                        ytr[:, c, :]
                        .rearrange("p (o q) -> p o q", o=1)
                        .broadcast_to([P, P, Q])
                    )
                    nc.vector.tensor_tensor(
                        a_pl[:], iotaT[:], y_bc, mybir.AluOpType.is_equal
                    )

                    # moving operand: bs = onehot(X)*V, per plane class
                    if plane_class(c) == "s":
                        # all-2x DVE builds, strided [P, x, q] B (slower MMs)
                        xeq = x_pool.tile([P, P, Q], FP16, tag="xeq")
                        x_bc = (
                            xtr[:, c, :]
                            .rearrange("p (o q) -> p o q", o=1)
                            .broadcast_to([P, P, Q])
                        )
                        nc.vector.tensor_tensor(
                            xeq[:], iotaT[:], x_bc, mybir.AluOpType.is_equal
                        )
                        bst = x_pool.tile([P, P, Q], FP16, tag="bst")
                        v_bc = (
                            vtr[:, c, :]
                            .rearrange("p (o q) -> p o q", o=1)
                            .broadcast_to([P, P, Q])
                        )
                        nc.vector.tensor_tensor(
                            bst[:], xeq[:], v_bc, mybir.AluOpType.mult
                        )
                        # ACT restride to [P, q, x] so the PE moving stream
                        # stays contiguous (ACT is the least-loaded engine)
                        bsc = c_pool.tile([P, Q, P], FP16, tag="bsc")
                        nc.scalar.copy(bsc[:], bst[:].rearrange("p x q -> p q x"))
                        rhs_sel = lambda q, _t=bsc: _t[:, q, :]
                    else:
                        chunks = []
                        for g, (q0, nq, ne) in enumerate(LS_CHUNKS):
                            bc_t = (b_pool if g % 2 == 0 else b_pool2).tile(
                                [P, ne], FP16, tag=f"b{ne}"
                            )
                            nc.gpsimd.local_scatter(
                                bc_t[:],
                                vtr[:, c, q0:q0 + nq],
                                idx16[:, c, q0:q0 + nq],
                                channels=P,
                                num_elems=ne,
                                num_idxs=nq,
                            )
                            chunks.append((q0, bc_t))

                        def rhs_sel(q, _ch=chunks):
                            for q0, t in reversed(_ch):
                                if q >= q0:
                                    return t[:, (q - q0) * P:(q - q0 + 1) * P]

                    acc = psum_pool.tile([P, P], F32)  # [y, x]
                    for q in range(Q):
                        nc.tensor.matmul(
                            acc[:],
                            a_pl[:, :, q],
                            rhs_sel(q),
                            start=(q == 0),
                            stop=(q == Q - 1),
                        )
                    nc.scalar.copy(osb_v[:, :, c], acc[:])

                nc.sync.dma_start(out[b], out_sb[:])

    nc.compile()
    return nc


_CACHED = {}


def _get_nc():
    if "nc" not in _CACHED:
        _CACHED["nc"] = build_nc()
    return _CACHED["nc"]


def make_in_maps(updates: np.ndarray, mask: np.ndarray):
    iota = np.broadcast_to(np.arange(P, dtype=np.float32), (P, P)).copy()
    jrow = np.empty(Q, dtype=np.int32)
    for q0, nq, _ne in LS_CHUNKS:
        jrow[q0:q0 + nq] = (np.arange(nq, dtype=np.int32)) * 128
    jq = np.broadcast_to(jrow, (P, Q)).copy()
    in_maps = []
    for i in range(N_CORES):
        sl = slice(i * B_PER_CORE, (i + 1) * B_PER_CORE)
        in_maps.append(
            {
                "updates": np.ascontiguousarray(
                    updates[sl].reshape(B_PER_CORE, HWF, C), dtype=np.float32
                ),
                "mask": np.ascontiguousarray(
                    mask[sl].reshape(B_PER_CORE, HWF, C), dtype=np.int32
                ),
                "iota": iota,
                "jq": jq,
            }
        )
    return in_maps


def kernel(updates: np.ndarray, mask: np.ndarray) -> np.ndarray:
    nc = _get_nc()
    in_maps = make_in_maps(updates, mask)
    res = run_bass_kernel_spmd(nc, in_maps, list(range(N_CORES)))
    parts = [
        res.results[i]["out"].reshape(B_PER_CORE, OH, OW, C).astype(np.float32)
        for i in range(N_CORES)
    ]
    return np.concatenate(parts, axis=0)


# revision 10
# speedup vs baseline: 1.2298x; 1.2298x over previous
"""MaxUnpooling2D scatter-add kernel for Trainium2 (8 NeuronCores).

Reference semantics (per batch b):
    y = mask // (OW*C); x = (mask // C) % OW; f = channel index c
    out[b, y, x, c] += updates[b, h, w, c]      (duplicates sum)

Strategy (pure data-parallel over batch; 2 batches per core):
  - One-hot matmul routing per (plane c, q-group): psum[y, x] += A_q.T @ B_q
    where A_q[i, y] = onehot(Y_i) (stationary) and B_q[i, x] = onehot(X_i)*V_i
    (moving). PSUM accumulates the 32 q-groups of a plane; duplicates sum.
  - The binding constraint is one-hot operand fabrication (8192 fp16 slots
    per plane) under a hardware DVFS throttle that tracks total engine
    activity. The work is spread across ALL four non-PE engines so none
    saturates (measured ~0.9-1.0 ms each on a ~1.23 ms wall):
      * A-tiles (all planes): DVE is_equal against a materialized iota in
        [P, y, q] layout (q innermost keeps the 2x packed mode; the strided
        LDWEIGHTS this causes overlaps under the matmuls).
      * B-tiles, 'g' planes (22 of every 32 channels): GPSIMD local_scatter
        of V into zeroed chunks at idx = (q-q0)*128 + X, chunked
        (14q,14q,4q)=(1792,1792,512) slots to amortize launch overhead;
        contiguous for the PE moving stream.
      * B-tiles, 's' planes (10 of every 32): DVE is_equal + mult (both 2x
        packed) in the strided [P, x, q] layout, then the ACT engine
        restrides to contiguous [P, q, x] (ACT is otherwise mostly idle;
        a strided PE moving stream would cost ~3x on the matmuls).
  - Output: accumulate all 128 planes of a batch into a [P(y), OW*C] fp16
    SBUF buffer (ACT evacuates psum[y, x] into column slot x*C + c), then a
    single contiguous 4 MB DMA per batch ([128, 32KB] rows). This replaces
    the v1 per-32-plane strided DMA that generated 131k x 128B packets.
    Output is fp16 on device; host upconverts to fp32 during unshard
    (values are already fp16-rounded on the way in, so this costs nothing).
  - Measured (core 0, traced): 1.228 ms vs 2.007 ms for the staged baseline
    under the same harness (1.739 ms untraced) -- ~39% faster. Per-op
    timings: is_equal [P,128,32] ~2.29 us (DVE 2x), local_scatter 1792-slot
    ~2.2 us, matmul p50 229 ns, ACT restride ~6.5 us. DVFS throttle fell
    from 73.8% to 43.4% of runtime once the DMA packet storm and GPSIMD
    saturation were removed.
  - Failed variants (measured): 3-class splits with strided PE moving
    operands (PE binds ~3x per matmul); eliminating the batch-boundary
    bubble via tr double-buffering + single out_sb buffer made EVERY op
    15-25% slower (more simultaneous engine overlap -> SBUF contention,
    1.51 ms). Engine-idle gaps on this part are partly load-shedding, not
    pure waste.
"""

import sys

sys.path.insert(0, "/opt/trn_rl_repo")

import numpy as np

import concourse.bacc as bacc
import concourse.tile as tile
from concourse import mybir, library_config
from concourse.bass_utils import run_bass_kernel_spmd

# Problem shape (hardcoded per contract)
B, H, W, C = 16, 64, 64, 128
OH, OW = 2 * H, 2 * W
N_CORES = 8
B_PER_CORE = B // N_CORES  # 2
HWF = H * W  # 4096
P = 128
Q = HWF // P  # 32 hw rows per partition
NCOL = Q * C  # 4096
NHALF = NCOL // 2  # decode in halves to save SBUF
QH = Q // 2
OWC = OW * C  # 16384

F32 = mybir.dt.float32
FP16 = mybir.dt.float16
I32 = mybir.dt.int32
I16 = mybir.dt.int16

# plane classes by c % 32, finely interleaved so no engine starves:
# 'g' GPSIMD local_scatter (contiguous B), 's' DVE 2x builds with strided B
# (slower MMs)
S_SET = frozenset({1, 4, 7, 10, 13, 16, 19, 22, 25, 28})

# local_scatter q-chunking: (q0, nq, num_elems); idx = (q - q0)*128 + X.
LS_CHUNKS = ((0, 14, 1792), (14, 14, 1792), (28, 4, 512))


def build_nc(s_set=S_SET):
    nc = bacc.Bacc("TRN2", target_bir_lowering=False, debug=False)

    upd = nc.declare_dram_parameter("updates", [B_PER_CORE, HWF, C], F32, isOutput=False)
    msk = nc.declare_dram_parameter("mask", [B_PER_CORE, HWF, C], I32, isOutput=False)
    iota_in = nc.declare_dram_parameter("iota", [P, P], F32, isOutput=False)
    jq_in = nc.declare_dram_parameter("jq", [P, Q], I32, isOutput=False)
    out = nc.declare_dram_parameter("out", [B_PER_CORE, OH, OWC], FP16, isOutput=True)

    def plane_class(c):
        return "s" if (c % 32) in s_set else "g"

    with tile.TileContext(nc) as tc:
        with (
            tc.tile_pool(name="const", bufs=1) as const_pool,
            tc.tile_pool(name="dec", bufs=1) as dec_pool,
            tc.tile_pool(name="tr", bufs=1) as tr_pool,
            tc.tile_pool(name="osb", bufs=2) as out_pool,
            tc.tile_pool(name="apool", bufs=2) as a_pool,
            tc.tile_pool(name="bpool", bufs=2) as b_pool,
            tc.tile_pool(name="bpool2", bufs=2) as b_pool2,
            tc.tile_pool(name="xpool", bufs=1) as x_pool,
            tc.tile_pool(name="cpool", bufs=2) as c_pool,
            tc.tile_pool(name="psum", bufs=8, space="PSUM") as psum_pool,
        ):
            nc.gpsimd.load_library(library_config.local_scatter)

            iota_f = const_pool.tile([P, P], F32)
            nc.sync.dma_start(iota_f[:], iota_in[:])
            jq = const_pool.tile([P, Q], I32)
            nc.sync.dma_start(jq[:], jq_in[:])
            # iotaT[p, y, q] = y (fp16, innermost step 1 -> DVE 2x packed)
            iotaT = const_pool.tile([P, P, Q], FP16)
            nc.vector.tensor_copy(
                iotaT[:],
                iota_f[:].rearrange("p (y o) -> p y o", o=1).broadcast_to([P, P, Q]),
            )

            for b in range(B_PER_CORE):
                # ---- load + decode batch b (in column-halves) ----
                ytr = tr_pool.tile([P, C, Q], FP16, tag="ytr")
                vtr = tr_pool.tile([P, C, Q], FP16, tag="vtr")
                xtr = tr_pool.tile([P, C, Q], FP16, tag="xtr")
                idx16 = tr_pool.tile([P, C, Q], I16, tag="idx16")
                for h in range(2):
                    qs = slice(h * QH, (h + 1) * QH)
                    cs = slice(h * NHALF, (h + 1) * NHALF)
                    u_f = dec_pool.tile([P, NHALF], F32, tag="uf")
                    nc.sync.dma_start(
                        u_f[:], upd[b].rearrange("(p q) c -> p (q c)", p=P)[:, cs]
                    )
                    m = dec_pool.tile([P, NHALF], I32, tag="m")
                    nc.sync.dma_start(
                        m[:], msk[b].rearrange("(p q) c -> p (q c)", p=P)[:, cs]
                    )
                    nc.scalar.copy(
                        vtr[:, :, qs], u_f[:].rearrange("p (q c) -> p c q", c=C)
                    )
                    yi = dec_pool.tile([P, NHALF], I32, tag="yi")
                    nc.vector.tensor_scalar(
                        yi[:], m[:], 14, None, mybir.AluOpType.logical_shift_right
                    )
                    nc.scalar.copy(
                        ytr[:, :, qs], yi[:].rearrange("p (q c) -> p c q", c=C)
                    )
                    xi = dec_pool.tile([P, NHALF], I32, tag="xi")
                    nc.vector.tensor_scalar(
                        xi[:],
                        m[:],
                        7,
                        127,
                        mybir.AluOpType.logical_shift_right,
                        mybir.AluOpType.bitwise_and,
                    )
                    nc.scalar.copy(
                        xtr[:, :, qs], xi[:].rearrange("p (q c) -> p c q", c=C)
                    )
                    # idx16[p, c, q] = X + (q%8)*128 (int16, for local_scatter)
                    nc.vector.scalar_tensor_tensor(
                        idx16[:, :, qs],
                        xi[:].rearrange("p (q c) -> p c q", c=C),
                        0,
                        jq[:, qs]
                        .rearrange("p (o q) -> p o q", o=1)
                        .broadcast_to([P, C, QH]),
                        mybir.AluOpType.add,
                        mybir.AluOpType.add,
                    )

                # [P(y), OW*C] fp16 accumulator for the whole batch output
                out_sb = out_pool.tile([P, OWC], FP16, tag="osb")
                osb_v = out_sb[:].rearrange("p (x c) -> p x c", c=C)

                for c in range(C):
                    # stationary: a[p, y, q] = (iotaT == Y) -- DVE 2x packed
                    a_pl = a_pool.tile([P, P, Q], FP16, tag="a")
                    y_bc = (
                        ytr[:, c, :]
                        .rearrange("p (o q) -> p o q", o=1)
                        .broadcast_to([P, P, Q])
                    )
                    nc.vector.tensor_tensor(
                        a_pl[:], iotaT[:], y_bc, mybir.AluOpType.is_equal
                    )

                    # moving operand: bs = onehot(X)*V, per plane class
                    if plane_class(c) == "s":
                        # all-2x DVE builds, strided [P, x, q] B (slower MMs)
                        xeq = x_pool.tile([P, P, Q], FP16, tag="xeq")
                        x_bc = (
                            xtr[:, c, :]
                            .rearrange("p (o q) -> p o q", o=1)
                            .broadcast_to([P, P, Q])
                        )
                        nc.vector.tensor_tensor(
                            xeq[:], iotaT[:], x_bc, mybir.AluOpType.is_equal
                        )
                        bst = x_pool.tile([P, P, Q], FP16, tag="bst")
                        v_bc = (
                            vtr[:, c, :]
                            .rearrange("p (o q) -> p o q", o=1)
                            .broadcast_to([P, P, Q])
                        )
                        nc.vector.tensor_tensor(
                            bst[:], xeq[:], v_bc, mybir.AluOpType.mult
                        )
                        # ACT restride to [P, q, x] so the PE moving stream
                        # stays contiguous (ACT is the least-loaded engine)
                        bsc = c_pool.tile([P, Q, P], FP16, tag="bsc")
                        nc.scalar.copy(bsc[:], bst[:].rearrange("p x q -> p q x"))
                        rhs_sel = lambda q, _t=bsc: _t[:, q, :]
                    else:
                        chunks = []
                        for g, (q0, nq, ne) in enumerate(LS_CHUNKS):
                            bc_t = (b_pool if g % 2 == 0 else b_pool2).tile(
                                [P, ne], FP16, tag=f"b{ne}"
                            )
                            nc.gpsimd.local_scatter(
                                bc_t[:],
                                vtr[:, c, q0:q0 + nq],
                                idx16[:, c, q0:q0 + nq],
                                channels=P,
                                num_elems=ne,
                                num_idxs=nq,
                            )
                            chunks.append((q0, bc_t))

                        def rhs_sel(q, _ch=chunks):
                            for q0, t in reversed(_ch):
                                if q >= q0:
                                    return t[:, (q - q0) * P:(q - q0 + 1) * P]

                    acc = psum_pool.tile([P, P], F32)  # [y, x]
                    for q in range(Q):
                        nc.tensor.matmul(
                            acc[:],
                            a_pl[:, :, q],
                            rhs_sel(q),
                            start=(q == 0),
                            stop=(q == Q - 1),
                        )
                    nc.scalar.copy(osb_v[:, :, c], acc[:])

                nc.sync.dma_start(out[b], out_sb[:])

    nc.compile()
    return nc


_CACHED = {}


def _get_nc():
    if "nc" not in _CACHED:
        _CACHED["nc"] = build_nc()
    return _CACHED["nc"]


def make_in_maps(updates: np.ndarray, mask: np.ndarray):
    iota = np.broadcast_to(np.arange(P, dtype=np.float32), (P, P)).copy()
    jrow = np.empty(Q, dtype=np.int32)
    for q0, nq, _ne in LS_CHUNKS:
        jrow[q0:q0 + nq] = (np.arange(nq, dtype=np.int32)) * 128
    jq = np.broadcast_to(jrow, (P, Q)).copy()
    in_maps = []
    for i in range(N_CORES):
        sl = slice(i * B_PER_CORE, (i + 1) * B_PER_CORE)
        in_maps.append(
            {
                "updates": np.ascontiguousarray(
                    updates[sl].reshape(B_PER_CORE, HWF, C), dtype=np.float32
                ),
                "mask": np.ascontiguousarray(
                    mask[sl].reshape(B_PER_CORE, HWF, C), dtype=np.int32
                ),
                "iota": iota,
                "jq": jq,
            }
        )
    return in_maps


def kernel(updates: np.ndarray, mask: np.ndarray) -> np.ndarray:
    nc = _get_nc()
    in_maps = make_in_maps(updates, mask)
    res = run_bass_kernel_spmd(nc, in_maps, list(range(N_CORES)))
    parts = [
        res.results[i]["out"].reshape(B_PER_CORE, OH, OW, C).astype(np.float32)
        for i in range(N_CORES)
    ]
    return np.concatenate(parts, axis=0)


# revision 11
# speedup vs baseline: 1.2434x; 1.0111x over previous
"""MaxUnpooling2D scatter-add kernel for Trainium2 (8 NeuronCores).

Reference semantics (per batch b):
    y = mask // (OW*C); x = (mask // C) % OW; f = channel index c
    out[b, y, x, c] += updates[b, h, w, c]      (duplicates sum)

Strategy (pure data-parallel over batch; 2 batches per core):
  - One-hot matmul routing per (plane c, q-group): psum[y, x] += A_q.T @ B_q
    where A_q[i, y] = onehot(Y_i) (stationary) and B_q[i, x] = onehot(X_i)*V_i
    (moving). PSUM accumulates the 32 q-groups of a plane; duplicates sum.
  - The binding constraint is one-hot operand fabrication (8192 fp16 slots
    per plane) under a hardware DVFS throttle that tracks total engine
    activity. The work is spread across ALL four non-PE engines so none
    saturates (measured ~0.9-1.0 ms each on a ~1.23 ms wall):
      * A-tiles (all planes): DVE is_equal against a materialized iota in
        [P, y, q] layout (q innermost keeps the 2x packed mode; the strided
        LDWEIGHTS this causes overlaps under the matmuls).
      * B-tiles, 'g' planes (22 of every 32 channels): GPSIMD local_scatter
        of V into zeroed chunks at idx = (q-q0)*128 + X, chunked
        (14q,14q,4q)=(1792,1792,512) slots to amortize launch overhead;
        contiguous for the PE moving stream.
      * B-tiles, 's' planes (10 of every 32): DVE is_equal + mult (both 2x
        packed) in the strided [P, x, q] layout, then the ACT engine
        restrides to contiguous [P, q, x] (ACT is otherwise mostly idle;
        a strided PE moving stream would cost ~3x on the matmuls).
  - Output: accumulate all 128 planes of a batch into a [P(y), OW*C] fp16
    SBUF buffer (ACT evacuates psum[y, x] into column slot x*C + c), then a
    single contiguous 4 MB DMA per batch ([128, 32KB] rows). This replaces
    the v1 per-32-plane strided DMA that generated 131k x 128B packets.
    Output is fp16 on device; host upconverts to fp32 during unshard
    (values are already fp16-rounded on the way in, so this costs nothing).
  - Measured (core 0, traced): 1.228 ms vs 2.007 ms for the staged baseline
    under the same harness (1.739 ms untraced) -- ~39% faster. Per-op
    timings: is_equal [P,128,32] ~2.29 us (DVE 2x), local_scatter 1792-slot
    ~2.2 us, matmul p50 229 ns, ACT restride ~6.5 us. DVFS throttle fell
    from 73.8% to 43.4% of runtime once the DMA packet storm and GPSIMD
    saturation were removed.
  - Failed variants (measured): 3-class splits with strided PE moving
    operands (PE binds ~3x per matmul); eliminating the batch-boundary
    bubble via tr double-buffering + single out_sb buffer made EVERY op
    15-25% slower (more simultaneous engine overlap -> SBUF contention,
    1.51 ms). Engine-idle gaps on this part are partly load-shedding, not
    pure waste.
"""

import sys

sys.path.insert(0, "/opt/trn_rl_repo")

import numpy as np

import concourse.bacc as bacc
import concourse.tile as tile
from concourse import mybir, library_config
from concourse.bass_utils import run_bass_kernel_spmd

# Problem shape (hardcoded per contract)
B, H, W, C = 16, 64, 64, 128
OH, OW = 2 * H, 2 * W
N_CORES = 8
B_PER_CORE = B // N_CORES  # 2
HWF = H * W  # 4096
P = 128
Q = HWF // P  # 32 hw rows per partition
NCOL = Q * C  # 4096
NHALF = NCOL // 4  # decode in quarter-columns to save SBUF
QH = Q // 4
OWC = OW * C  # 16384

F32 = mybir.dt.float32
FP16 = mybir.dt.float16
I32 = mybir.dt.int32
I16 = mybir.dt.int16

# plane classes by c % 32, finely interleaved so no engine starves:
# 'g' GPSIMD local_scatter (contiguous B), 's' DVE 2x builds with strided B
# (slower MMs)
S_SET = frozenset({1, 4, 7, 10, 13, 16, 19, 22, 25, 28})

# local_scatter q-chunking: (q0, nq, num_elems); idx = (q - q0)*128 + X.
LS_CHUNKS = ((0, 14, 1792), (14, 14, 1792), (28, 4, 512))


def build_nc(s_set=S_SET):
    nc = bacc.Bacc("TRN2", target_bir_lowering=False, debug=False)

    upd = nc.declare_dram_parameter("updates", [B_PER_CORE, HWF, C], F32, isOutput=False)
    msk = nc.declare_dram_parameter("mask", [B_PER_CORE, HWF, C], I32, isOutput=False)
    iota_in = nc.declare_dram_parameter("iota", [P, P], F32, isOutput=False)
    jq_in = nc.declare_dram_parameter("jq", [P, Q], I32, isOutput=False)
    out = nc.declare_dram_parameter("out", [B_PER_CORE, OH, OWC], FP16, isOutput=True)

    def plane_class(c):
        return "s" if (c % 32) in s_set else "g"

    with tile.TileContext(nc) as tc:
        with (
            tc.tile_pool(name="const", bufs=1) as const_pool,
            tc.tile_pool(name="dec", bufs=1) as dec_pool,
            tc.tile_pool(name="tr", bufs=1) as tr_pool,
            tc.tile_pool(name="osb", bufs=2) as out_pool,
            tc.tile_pool(name="apool", bufs=2) as a_pool,
            tc.tile_pool(name="bpool", bufs=3) as b_pool,
            tc.tile_pool(name="bpool2", bufs=3) as b_pool2,
            tc.tile_pool(name="xpool", bufs=1) as x_pool,
            tc.tile_pool(name="cpool", bufs=2) as c_pool,
            tc.tile_pool(name="psum", bufs=8, space="PSUM") as psum_pool,
        ):
            nc.gpsimd.load_library(library_config.local_scatter)

            iota_f = const_pool.tile([P, P], F32)
            nc.sync.dma_start(iota_f[:], iota_in[:])
            jq = const_pool.tile([P, Q], I32)
            nc.sync.dma_start(jq[:], jq_in[:])
            # iotaT[p, y, q] = y (fp16, innermost step 1 -> DVE 2x packed)
            iotaT = const_pool.tile([P, P, Q], FP16)
            nc.vector.tensor_copy(
                iotaT[:],
                iota_f[:].rearrange("p (y o) -> p y o", o=1).broadcast_to([P, P, Q]),
            )

            for b in range(B_PER_CORE):
                # ---- load + decode batch b (in column-halves) ----
                ytr = tr_pool.tile([P, C, Q], FP16, tag="ytr")
                vtr = tr_pool.tile([P, C, Q], FP16, tag="vtr")
                xtr = tr_pool.tile([P, C, Q], FP16, tag="xtr")
                idx16 = tr_pool.tile([P, C, Q], I16, tag="idx16")
                for h in range(4):
                    qs = slice(h * QH, (h + 1) * QH)
                    cs = slice(h * NHALF, (h + 1) * NHALF)
                    u_f = dec_pool.tile([P, NHALF], F32, tag="uf")
                    nc.sync.dma_start(
                        u_f[:], upd[b].rearrange("(p q) c -> p (q c)", p=P)[:, cs]
                    )
                    m = dec_pool.tile([P, NHALF], I32, tag="m")
                    nc.sync.dma_start(
                        m[:], msk[b].rearrange("(p q) c -> p (q c)", p=P)[:, cs]
                    )
                    nc.scalar.copy(
                        vtr[:, :, qs], u_f[:].rearrange("p (q c) -> p c q", c=C)
                    )
                    yi = dec_pool.tile([P, NHALF], I32, tag="yi")
                    nc.vector.tensor_scalar(
                        yi[:], m[:], 14, None, mybir.AluOpType.logical_shift_right
                    )
                    nc.scalar.copy(
                        ytr[:, :, qs], yi[:].rearrange("p (q c) -> p c q", c=C)
                    )
                    xi = dec_pool.tile([P, NHALF], I32, tag="xi")
                    nc.vector.tensor_scalar(
                        xi[:],
                        m[:],
                        7,
                        127,
                        mybir.AluOpType.logical_shift_right,
                        mybir.AluOpType.bitwise_and,
                    )
                    nc.scalar.copy(
                        xtr[:, :, qs], xi[:].rearrange("p (q c) -> p c q", c=C)
                    )
                    # idx16[p, c, q] = X + (q%8)*128 (int16, for local_scatter)
                    nc.vector.scalar_tensor_tensor(
                        idx16[:, :, qs],
                        xi[:].rearrange("p (q c) -> p c q", c=C),
                        0,
                        jq[:, qs]
                        .rearrange("p (o q) -> p o q", o=1)
                        .broadcast_to([P, C, QH]),
                        mybir.AluOpType.add,
                        mybir.AluOpType.add,
                    )

                # [P(y), OW*C] fp16 accumulator for the whole batch output
                out_sb = out_pool.tile([P, OWC], FP16, tag="osb")
                osb_v = out_sb[:].rearrange("p (x c) -> p x c", c=C)

                for c in range(C):
                    # stationary: a[p, y, q] = (iotaT == Y) -- DVE 2x packed
                    a_pl = a_pool.tile([P, P, Q], FP16, tag="a")
                    y_bc = (
                        ytr[:, c, :]
                        .rearrange("p (o q) -> p o q", o=1)
                        .broadcast_to([P, P, Q])
                    )
                    nc.vector.tensor_tensor(
                        a_pl[:], iotaT[:], y_bc, mybir.AluOpType.is_equal
                    )

                    # moving operand: bs = onehot(X)*V, per plane class
                    if plane_class(c) == "s":
                        # all-2x DVE builds, strided [P, x, q] B (slower MMs)
                        xeq = x_pool.tile([P, P, Q], FP16, tag="xeq")
                        x_bc = (
                            xtr[:, c, :]
                            .rearrange("p (o q) -> p o q", o=1)
                            .broadcast_to([P, P, Q])
                        )
                        nc.vector.tensor_tensor(
                            xeq[:], iotaT[:], x_bc, mybir.AluOpType.is_equal
                        )
                        bst = x_pool.tile([P, P, Q], FP16, tag="bst")
                        v_bc = (
                            vtr[:, c, :]
                            .rearrange("p (o q) -> p o q", o=1)
                            .broadcast_to([P, P, Q])
                        )
                        nc.vector.tensor_tensor(
                            bst[:], xeq[:], v_bc, mybir.AluOpType.mult
                        )
                        # ACT restride to [P, q, x] so the PE moving stream
                        # stays contiguous (ACT is the least-loaded engine)
                        bsc = c_pool.tile([P, Q, P], FP16, tag="bsc")
                        nc.scalar.copy(bsc[:], bst[:].rearrange("p x q -> p q x"))
                        rhs_sel = lambda q, _t=bsc: _t[:, q, :]
                    else:
                        chunks = []
                        for g, (q0, nq, ne) in enumerate(LS_CHUNKS):
                            bc_t = (b_pool if g % 2 == 0 else b_pool2).tile(
                                [P, ne], FP16, tag=f"b{ne}"
                            )
                            nc.gpsimd.local_scatter(
                                bc_t[:],
                                vtr[:, c, q0:q0 + nq],
                                idx16[:, c, q0:q0 + nq],
                                channels=P,
                                num_elems=ne,
                                num_idxs=nq,
                            )
                            chunks.append((q0, bc_t))

                        def rhs_sel(q, _ch=chunks):
                            for q0, t in reversed(_ch):
                                if q >= q0:
                                    return t[:, (q - q0) * P:(q - q0 + 1) * P]

                    acc = psum_pool.tile([P, P], F32)  # [y, x]
                    for q in range(Q):
                        nc.tensor.matmul(
                            acc[:],
                            a_pl[:, :, q],
                            rhs_sel(q),
                            start=(q == 0),
                            stop=(q == Q - 1),
                        )
                    nc.scalar.copy(osb_v[:, :, c], acc[:])

                nc.sync.dma_start(out[b], out_sb[:])

    nc.compile()
    return nc


_CACHED = {}


def _get_nc():
    if "nc" not in _CACHED:
        _CACHED["nc"] = build_nc()
    return _CACHED["nc"]


def make_in_maps(updates: np.ndarray, mask: np.ndarray):
    iota = np.broadcast_to(np.arange(P, dtype=np.float32), (P, P)).copy()
    jrow = np.empty(Q, dtype=np.int32)
    for q0, nq, _ne in LS_CHUNKS:
        jrow[q0:q0 + nq] = (np.arange(nq, dtype=np.int32)) * 128
    jq = np.broadcast_to(jrow, (P, Q)).copy()
    in_maps = []
    for i in range(N_CORES):
        sl = slice(i * B_PER_CORE, (i + 1) * B_PER_CORE)
        in_maps.append(
            {
                "updates": np.ascontiguousarray(
                    updates[sl].reshape(B_PER_CORE, HWF, C), dtype=np.float32
                ),
                "mask": np.ascontiguousarray(
                    mask[sl].reshape(B_PER_CORE, HWF, C), dtype=np.int32
                ),
                "iota": iota,
                "jq": jq,
            }
        )
    return in_maps


def kernel(updates: np.ndarray, mask: np.ndarray) -> np.ndarray:
    nc = _get_nc()
    in_maps = make_in_maps(updates, mask)
    res = run_bass_kernel_spmd(nc, in_maps, list(range(N_CORES)))
    parts = [
        res.results[i]["out"].reshape(B_PER_CORE, OH, OW, C).astype(np.float32)
        for i in range(N_CORES)
    ]
    return np.concatenate(parts, axis=0)
